# revision 1
# baseline (speedup 1.0000x reference)
"""Trainium2 Bass kernel for nn_KernelDensityLoss (KDE softmax loss).

Math: the reference's O(B^2*D) pairwise log-prob matrix collapses to
per-class sufficient statistics.  For row i and class c,

  sums[i,c] = sum_{n in c} lp[i,n]
            = -0.5*(M*const + (M*sq[i] + Ssq[c] - 2*x_i.S_c)/var)

with S_c = sum of class-c embeddings [D], Ssq[c] = sum of squared norms,
sq[i] = ||x_i||^2.  The -0.5*const shift is identical for the own-class
(leave-one-out) and other-class branches and cancels in
logsumexp(row) - own, so the kernel only computes

  A[i,c] = M*sq[i] + Ssq[c] - 2*G[i,c]        (G = X @ S^T)
  P[i,c] = -0.5*A[i,c] / (var*m_c)            (m_c = M-1 own class, M else)
  loss   = sum_i relu(logsumexp_c P[i,c] - P[i,own])

Distribution: B=7168 rows sharded 896/core across 8 NeuronCores.  Each
core computes partial class stats with PE matmuls against the one-hot
class matrix (lhsT = x_tile -> stats come out directly with D on the
partition axis, no transposes), AllGather + local sum combines them
(lower latency floor than AllReduce), then each core evaluates its own
896 rows and emits a partial loss scalar; the host sums 8 scalars.
"""

import numpy as np

import concourse.bass as bass
import concourse.bacc as bacc
import concourse.mybir as mybir
import concourse.tile as tile
from concourse.bass_utils import run_bass_kernel_spmd

B = 7168      # total rows
C = 7         # classes
M = 1024      # rows per class
D = 256       # embedding dim
NCORES = 8
R = B // NCORES          # 896 rows per core
T = R // 128             # 7 row-tiles of 128 per core

F32 = mybir.dt.float32
AX = mybir.AxisListType
AF = mybir.ActivationFunctionType
ALU = mybir.AluOpType

# stats layout (free dim of the [128, SW] stats tile):
#   cols 0:7    S half0  (class sums for d in [0,128))
#   cols 7:14   S half1  (class sums for d in [128,256))
#   row0 14:21  Ssq row  (per-class sum of squared norms)
SW = 24


def build_program():
    nc = bacc.Bacc(
        "TRN2",
        target_bir_lowering=False,
        debug=False,
        enable_asserts=True,
        num_devices=NCORES,
    )

    x_d = nc.dram_tensor("x", [R, D], F32, kind="ExternalInput")
    xf_d = nc.dram_tensor("xf", [B, D], F32, kind="ExternalInput")
    xt_d = nc.dram_tensor("xt", [D, R], F32, kind="ExternalInput")
    y_d = nc.dram_tensor("y", [R, C], F32, kind="ExternalInput")
    yf_d = nc.dram_tensor("yf", [B, C], F32, kind="ExternalInput")
    consts_d = nc.dram_tensor("consts", [128, 4], F32, kind="ExternalInput")
    ident_d = nc.dram_tensor("ident", [128, 128], F32, kind="ExternalInput")
    out_d = nc.dram_tensor("loss_part", [1, 1], F32, kind="ExternalOutput")
    TF = B // 128  # 56 tiles over the full batch

    with tile.TileContext(nc) as tc:
        with (
            tc.tile_pool(name="persist", bufs=1) as pp,
            tc.tile_pool(name="xtiles", bufs=4) as px,
            tc.tile_pool(name="scratch", bufs=2) as ps,
            tc.tile_pool(name="chunk", bufs=2) as pc,
            tc.tile_pool(name="psum_stat", bufs=1, space="PSUM") as qstat,
            tc.tile_pool(name="psum_p", bufs=2, space="PSUM") as qp,
            tc.tile_pool(name="dram", bufs=1, space="DRAM") as pd,
        ):
            # ---- persistent tiles ----
            xt0 = pp.tile([128, R], F32, tag="xt0")      # d in [0,128)
            xt1 = pp.tile([128, R], F32, tag="xt1")      # d in [128,256)
            ytile = pp.tile([128, T, C], F32, tag="y")   # own-shard mask [p, t, c]
            yftile = pp.tile([128, TF, C], F32, tag="yf")  # full one-hot [p, g, c]
            consts = pp.tile([128, 4], F32, tag="consts")
            ident = pp.tile([128, 128], F32, tag="ident")
            xfb = pp.tile([128, TF, D], F32, tag="xfb")  # full batch, tiled
            xsb = pp.tile([128, TF, D], F32, tag="xsb")  # its squares
            xb = pp.tile([128, T, D], F32, tag="xb")     # own shard
            xbs = pp.tile([128, T, D], F32, tag="xbs")
            sq = pp.tile([128, T], F32, tag="sq")        # own-shard ||x||^2
            b_oth = pp.tile([128, T], F32, tag="b_oth")
            sA = pp.tile([7, 512], F32, tag="sA")
            sB = pp.tile([7, 512], F32, tag="sB")
            st7 = pp.tile([7, 256], F32, tag="st7")
            s2 = pp.tile([7, 256], F32, tag="s2")
            s2h = pp.tile([128, 2 * C], F32, tag="s2h")
            ssq_sb = pp.tile([1, C], F32, tag="ssq_sb")
            shsc = pp.tile([128, 2 * C], F32, tag="shsc")
            accL = pp.tile([128, T], F32, tag="accL")
            accT = pp.tile([128, 1], F32, tag="accT")
            ones_row = pp.tile([1, 128], F32, tag="ones_row")
            ones_col = pp.tile([128, 1], F32, tag="ones_col")
            out_s = pp.tile([1, 1], F32, tag="out_s")

            psA = qstat.tile([7, 512], F32, tag="psA")
            psB = qstat.tile([7, 512], F32, tag="psB")
            ps_ssq = qstat.tile([1, C], F32, tag="ps_ssq")
            ploss = qstat.tile([1, 1], F32, tag="ploss")

            # ---- loads (few wide DMAs; per-partition packets are 1KB) ----
            nc.sync.dma_start(out=consts[:], in_=consts_d[:, :])
            nc.sync.dma_start(out=ident[:], in_=ident_d[:, :])
            nc.sync.dma_start(
                out=ytile[:],
                in_=y_d.ap().rearrange("(t p) c -> p t c", p=128),
            )
            nc.sync.dma_start(
                out=yftile[:],
                in_=yf_d.ap().rearrange("(t p) c -> p t c", p=128),
            )
            for h in range(T):
                lo, hi = h * 128, (h + 1) * 128
                nc.sync.dma_start(out=xt0[:, lo:hi], in_=xt_d[0:128, lo:hi])
                nc.sync.dma_start(out=xt1[:, lo:hi], in_=xt_d[128:256, lo:hi])
            xf_r = xf_d.ap().rearrange("(a p) d -> p a d", p=128)
            for j in range(8):
                nc.sync.dma_start(out=xfb[:, 7 * j:7 * j + 7, :],
                                  in_=xf_r[:, 7 * j:7 * j + 7, :])
            nc.sync.dma_start(out=xb[:],
                              in_=x_d.ap().rearrange("(t p) d -> p t d", p=128))

            nc.vector.memset(ones_row[:], 1.0)
            nc.vector.memset(ones_col[:], 1.0)

            # ---- own-shard row norms (per-row bias) ----
            nc.vector.tensor_mul(xbs[:], xb[:], xb[:])
            nc.vector.reduce_sum(sq[:], xbs[:], axis=AX.X)
            nc.scalar.activation(b_oth[:], sq[:], AF.Copy, bias=0.0, scale=consts[:, 2:3])

            # ---- full-batch squares: 3 wide ops split across engines ----
            nc.scalar.activation(xsb[:, 0:18, :], xfb[:, 0:18, :], AF.Square,
                                 bias=0.0, scale=1.0)
            nc.vector.tensor_mul(xsb[:, 18:38, :], xfb[:, 18:38, :], xfb[:, 18:38, :])
            nc.gpsimd.tensor_mul(xsb[:, 38:56, :], xfb[:, 38:56, :], xfb[:, 38:56, :])

            # ---- class stats: inverted matmuls, 512-wide moving, pair-batched.
            # lhsT = one-hot class column (constant within a 1024-row class, and
            # tile pairs never straddle classes), so one 7-col stationary load
            # covers 512 moving columns = two row-tiles. psA/psB halves hold
            # even/odd-tile partial sums; summed after evacuation. ----
            for j in range(TF // 2):
                g = 2 * j
                y_g = yftile[:, g, :]
                st = (j == 0)
                sp = (j == TF // 2 - 1)
                nc.tensor.matmul(psA[:], lhsT=y_g, rhs=xfb[:, g:g + 2, :],
                                 start=st, stop=sp)
                nc.tensor.matmul(psB[:], lhsT=y_g, rhs=xsb[:, g:g + 2, :],
                                 start=st, stop=sp)

            nc.scalar.copy(sA[:], psA[:])
            nc.scalar.copy(sB[:], psB[:])
            nc.vector.tensor_add(st7[:], sA[:, 0:256], sA[:, 256:512])
            nc.vector.tensor_add(s2[:], sB[:, 0:256], sB[:, 256:512])

            # transpose S.T [7,256] -> Sh [128,14] (and squares) via PE
            for h in range(2):
                tp = qp.tile([128, C], F32, tag="pP")
                nc.tensor.transpose(tp[:], st7[:, 128 * h:128 * h + 128],
                                    ident[0:C, 0:C])
                # shsc = -2 * S, folded into the evacuation
                nc.scalar.activation(shsc[:, C * h:C * h + C], tp[:], AF.Copy,
                                     bias=0.0, scale=-2.0)
                tq = qp.tile([128, C], F32, tag="pP")
                nc.tensor.transpose(tq[:], s2[:, 128 * h:128 * h + 128],
                                    ident[0:C, 0:C])
                nc.scalar.copy(s2h[:, C * h:C * h + C], tq[:])

            # Ssq row [1,7] = column sums of S2 halves
            nc.tensor.matmul(ps_ssq[:], lhsT=ones_col[:], rhs=s2h[:, 0:C],
                             start=True, stop=False)
            nc.tensor.matmul(ps_ssq[:], lhsT=ones_col[:], rhs=s2h[:, C:2 * C],
                             start=False, stop=True)
            nc.scalar.copy(ssq_sb[:], ps_ssq[:])

            # ---- phase 3: per-row loss ----
            for u in range(T):
                lo, hi = u * 128, (u + 1) * 128
                pP = qp.tile([128, C], F32, tag="pP")
                # pP = -2*G + Ssq  (c-dependent part of A)
                nc.tensor.matmul(pP[:], lhsT=xt0[:, lo:hi], rhs=shsc[:, 0:C],
                                 start=True, stop=False)
                nc.tensor.matmul(pP[:], lhsT=xt1[:, lo:hi], rhs=shsc[:, C:2 * C],
                                 start=False, stop=False)
                nc.tensor.matmul(pP[:], lhsT=ones_row[:], rhs=ssq_sb[0:1, 0:C],
                                 start=False, stop=True)

                # P_oth for all 7 columns; the own-class (leave-one-out)
                # value is an exact rescale: P_own = P_oth * M/(M-1), so the
                # select fuses into one multiply-add against the one-hot mask.
                p_oth = pc.tile([128, C], F32, tag="p_oth")
                nc.scalar.activation(p_oth[:], pP[:], AF.Identity,
                                     bias=b_oth[:, u:u + 1], scale=consts[:, 0:1])

                mask_u = ytile[:, u, :]
                # scr7raw = mask * p_oth  (only own column nonzero)
                scr7 = pc.tile([128, C], F32, tag="scr7")
                nc.vector.tensor_tensor(scr7[:], p_oth[:], mask_u, op=ALU.mult)
                # own value (pre-LOO): P_oth[own] = row-sum of scr7raw
                own_raw = pc.tile([128, 1], F32, tag="own_raw")
                nc.vector.reduce_sum(own_raw[:], scr7[:], axis=AX.X)
                # p_fin: own column scaled by M/(M-1) (the exact LOO value)
                sc2 = pc.tile([128, C], F32, tag="sc2")
                nc.vector.tensor_scalar_mul(sc2[:], scr7[:], 1.0 / (M - 1))
                p_fin = pc.tile([128, C], F32, tag="p_fin")
                nc.vector.tensor_add(p_fin[:], p_oth[:], sc2[:])

                nmx = pc.tile([128, 1], F32, tag="nmx")
                nc.vector.tensor_reduce(
                    out=nmx[:], in_=p_fin[:], axis=AX.X, op=ALU.max, negate=True
                )

                ex = pc.tile([128, C], F32, tag="ex")
                se = pc.tile([128, 1], F32, tag="se")
                nc.scalar.activation(ex[:], p_fin[:], AF.Exp,
                                     bias=nmx[:], scale=1.0, accum_out=se[:])
                lnse = pc.tile([128, 1], F32, tag="lnse")
                nc.scalar.activation(lnse[:], se[:], AF.Ln)

                # L = (lnse - nmx) - M/(M-1)*own_raw ; accL[:,u] = relu(L)
                s1 = pc.tile([128, 1], F32, tag="s1")
                nc.vector.tensor_sub(s1[:], lnse[:], nmx[:])
                ot = pc.tile([128, 1], F32, tag="ot")
                nc.vector.tensor_scalar_mul(ot[:], own_raw[:], -float(M) / (M - 1))
                l_u = pc.tile([128, 1], F32, tag="l_u")
                nc.vector.tensor_add(l_u[:], s1[:], ot[:])
                nc.vector.tensor_scalar_max(accL[:, u:u + 1], l_u[:], 0.0)

            # ---- reduce to scalar ----
            nc.vector.reduce_sum(accT[:], accL[:], axis=AX.X)
            nc.tensor.matmul(ploss[:], lhsT=accT[:], rhs=ones_col[:],
                             start=True, stop=True)
            nc.scalar.copy(out_s[:], ploss[:])
            nc.sync.dma_start(out=out_d[:, :], in_=out_s[:])

    nc.compile()
    return nc


_NC_CACHE = None


def _get_nc():
    global _NC_CACHE
    if _NC_CACHE is None:
        _NC_CACHE = build_program()
    return _NC_CACHE


def make_in_maps(embeddings, variance):
    X = np.ascontiguousarray(np.asarray(embeddings, dtype=np.float32))
    assert X.shape == (B, D), X.shape
    var = float(np.asarray(variance))

    labels = np.repeat(np.arange(C), M)  # reference ignores `target`
    Yfull = np.zeros((B, C), np.float32)
    Yfull[np.arange(B), labels] = 1.0

    consts = np.zeros((128, 4), np.float32)
    consts[:, 0] = -0.5 / (var * M)
    consts[:, 1] = -0.5 / (var * (M - 1))
    consts[:, 2] = -0.5 / var
    consts[:, 3] = -0.5 * M / (var * (M - 1))

    in_maps = []
    for k in range(NCORES):
        s = slice(k * R, (k + 1) * R)
        in_maps.append({
            "x": X[s],
            "xf": X,
            "xt": np.ascontiguousarray(X[s].T),
            "y": np.ascontiguousarray(Yfull[s]),
            "yf": Yfull,
            "consts": consts,
            "ident": np.eye(128, dtype=np.float32),
        })
    return in_maps


def kernel(embeddings, target, variance):
    del target  # labels are balanced & class-sorted by construction (as in reference)
    nc = _get_nc()
    in_maps = make_in_maps(embeddings, variance)
    res = run_bass_kernel_spmd(nc, in_maps, list(range(NCORES)))
    total = 0.0
    for k in range(NCORES):
        total += float(res.results[k]["loss_part"][0, 0])
    return np.float32(total)



# revision 23
# speedup vs baseline: 1.1133x; 1.1133x over previous
"""Trainium2 Bass kernel for nn_KernelDensityLoss (KDE softmax loss).

Math: the reference's O(B^2*D) pairwise log-prob matrix collapses to
per-class sufficient statistics.  For row i and class c,

  A[i,c] = M*sq[i] + Ssq[c] - 2*G[i,c]     (G = X @ S^T, sq = ||x_i||^2,
                                            S_c = class sum, Ssq_c = class
                                            sum of squared norms)
  P[i,c] = -0.5*A[i,c] / (var*m_c)         (m_c = M-1 own class, M else)
  loss   = sum_i relu(logsumexp_c P[i,c] - P[i,own])

The Gaussian normalisation constant cancels in logsumexp - own.  With
c0 = -0.5/(var*M), the kernel computes q[i,c] = c0*(Ssq_c - 2*G[i,c])
(the per-row M*sq[i] term is NOT added: a per-row additive constant K
cancels in z_c = P_c - (M/(M-1))*P_own except for a -c0*K/(M-1)
residue, which folds into the per-row exp bias):

  z_c   = q_c + b2_i,   b2_i = -(M/(M-1))*own_q_i - c0*M*sq_i/(M-1)
  se    = sum_c exp(z_c); the own column's term is replaced by its
          exact value 1 via  se += 1 - exp(z_own)  (z_own recomputed
          batched);  L_i = relu(ln(se)).

Distribution (data-parallel, per the sharding hint): B=7168 rows are
sharded 896/core across 8 NeuronCores.  Each core computes PARTIAL class
stats over its own 7 row-tiles only (x-stationary matmuls emit S^T
directly in [d, c] layout; a 1-column `sq` stationary emits Ssq with no
squares matmul), packs S0|S1|Ssq into one [128, 21] fp32 buffer, and a
single AllGather + 3-add local tree combines the 8 partials.  Phase 3
then runs on the core's own 896 rows only.

All Exp calls are batched before the single Ln so the activation table
is swapped once, not per row-tile.  Embeddings travel in bf16 (matmul
inputs only; accumulation fp32; the Ssq path is fully fp32).  bf16
rounding of per-row quantities cancels in z up to a 1/(M-1) factor.
NOTE: tensor_tensor_reduce is avoided (crashes this runtime);
scalar_tensor_tensor(+accum_out) replaces it.
"""

import numpy as np

import concourse.bass as bass
import concourse.bacc as bacc
import concourse.mybir as mybir
import concourse.tile as tile
from concourse.bass_utils import run_bass_kernel_spmd

B = 7168      # total rows
C = 7         # classes
M = 1024      # rows per class
D = 256       # embedding dim
NCORES = 8
R = B // NCORES          # 896 rows per core
T = R // 128             # 7 row-tiles of 128 per core

F32 = mybir.dt.float32
BF16 = mybir.dt.bfloat16
AX = mybir.AxisListType
AF = mybir.ActivationFunctionType
ALU = mybir.AluOpType

SW = 3 * C  # packed stats width: S half0 | S half1 | Ssq row


def build_program():
    nc = bacc.Bacc(
        "TRN2",
        target_bir_lowering=False,
        debug=False,
        enable_asserts=True,
        num_devices=NCORES,
    )

    xb_d = nc.dram_tensor("xb", [R, D], BF16, kind="ExternalInput")
    xt_d = nc.dram_tensor("xt", [D, R], BF16, kind="ExternalInput")
    y_d = nc.dram_tensor("y", [R, C], F32, kind="ExternalInput")
    yb_d = nc.dram_tensor("yb", [R, C], BF16, kind="ExternalInput")
    consts_d = nc.dram_tensor("consts", [128, 4], F32, kind="ExternalInput")
    out_d = nc.dram_tensor("loss_part", [1, 1], F32, kind="ExternalOutput")

    with tile.TileContext(nc) as tc:
        with (
            tc.tile_pool(name="persist", bufs=1) as pp,
            tc.tile_pool(name="sqscratch", bufs=2) as pq,
            tc.tile_pool(name="chunk", bufs=2) as pc,
            tc.tile_pool(name="dram", bufs=1, space="DRAM") as pd,
        ):
            # ---- persistent tiles ----
            xb = pp.tile([128, T, D], BF16, tag="xb")      # own shard, row-major
            xt0 = pp.tile([128, R], BF16, tag="xt0")       # d in [0,128)
            xt1 = pp.tile([128, R], BF16, tag="xt1")       # d in [128,256)
            ytile = pp.tile([128, T, C], F32, tag="y")     # one-hot mask
            ybt = pp.tile([128, T, C], BF16, tag="yb")
            consts = pp.tile([128, 4], F32, tag="consts")
            sq = pp.tile([128, T], F32, tag="sq")          # ||x_i||^2 fp32
            b_t2 = pp.tile([128, T], F32, tag="b_t2")      # sq*M*c0/(M-1)
            pack = pp.tile([128, SW], F32, tag="pack")     # local partial stats
            gath = pp.tile([128, NCORES, SW], F32, tag="gath")
            stat = pp.tile([128, SW], F32, tag="stat")     # summed stats
            shsc = pp.tile([128, 2 * C], BF16, tag="shsc")  # -2*c0*S^T, bf16
            ssqb = pp.tile([128, C], F32, tag="ssqb")      # c0*Ssq broadcast
            ones_row = pp.tile([1, 128], F32, tag="ones_row")
            ones_col = pp.tile([128, 1], F32, tag="ones_col")
            q_all = pp.tile([128, T, C], F32, tag="q_all")   # c0*(Ssq-2G)
            own_all = pp.tile([128, T], F32, tag="own_all")  # own_q per tile
            b2_all = pp.tile([128, T], F32, tag="b2_all")
            zo_all = pp.tile([128, T], F32, tag="zo_all")
            eo_all = pp.tile([128, T], F32, tag="eo_all")
            se_all = pp.tile([128, T], F32, tag="se_all")
            se_fix = pp.tile([128, T], F32, tag="se_fix")
            lnse = pp.tile([128, T], F32, tag="lnse")
            accL = pp.tile([128, T], F32, tag="accL")
            accT = pp.tile([128, 1], F32, tag="accT")
            out_s = pp.tile([1, 1], F32, tag="out_s")

            cc_in = pd.tile([128, SW], F32, tag="cc_in")
            cc_out = pd.tile([128 * NCORES, SW], F32, tag="cc_out",
                             addr_space="Shared")

            # ---- loads ----
            nc.sync.dma_start(out=consts[:], in_=consts_d[:, :])
            nc.sync.dma_start(out=ytile[:],
                              in_=y_d.ap().rearrange("(t p) c -> p t c", p=128))
            nc.sync.dma_start(out=ybt[:],
                              in_=yb_d.ap().rearrange("(t p) c -> p t c", p=128))
            nc.sync.dma_start(out=xb[:],
                              in_=xb_d.ap().rearrange("(t p) d -> p t d", p=128))
            nc.sync.dma_start(out=xt0[:], in_=xt_d[0:128, :])
            nc.sync.dma_start(out=xt1[:], in_=xt_d[128:256, :])

            nc.vector.memset(ones_row[:], 1.0)
            nc.gpsimd.memset(ones_col[:], 1.0)
            nc.gpsimd.memset(pack[:, 2 * C:3 * C], 0.0)

            # ---- row norms sq (split scalar/vector), fp32 accumulate ----
            for t in range(T):
                scr = pq.tile([128, D], F32, tag="sqscr")
                if t < 4:
                    nc.scalar.activation(scr[:], xb[:, t, :], AF.Square,
                                         bias=0.0, scale=1.0,
                                         accum_out=sq[:, t:t + 1])
                else:
                    nc.vector.tensor_mul(scr[:], xb[:, t, :], xb[:, t, :])
                    nc.vector.reduce_sum(sq[:, t:t + 1], scr[:], axis=AX.X)
            nc.vector.tensor_scalar_mul(b_t2[:], sq[:], consts[:, 2:3])

            # ---- partial class stats over own 7 tiles ----
            with tc.tile_pool(name="psum_stat", bufs=1, space="PSUM") as qstat:
                psS0 = qstat.tile([128, C], F32, tag="psS0")
                psS1 = qstat.tile([128, C], F32, tag="psS1")
                ps_ssq = qstat.tile([1, C], F32, tag="ps_ssq")
                for t in range(T):
                    st = (t == 0)
                    sp = (t == T - 1)
                    nc.tensor.matmul(psS0[:], lhsT=xb[:, t, 0:128],
                                     rhs=ybt[:, t, :], start=st, stop=sp)
                    nc.tensor.matmul(psS1[:], lhsT=xb[:, t, 128:256],
                                     rhs=ybt[:, t, :], start=st, stop=sp)
                    nc.tensor.matmul(ps_ssq[:], lhsT=sq[:, t:t + 1],
                                     rhs=ytile[:, t, :], start=st, stop=sp)

                # pack partial stats [128, 21] = S0 | S1 | Ssq(row0)
                nc.scalar.copy(pack[:, 0:C], psS0[:])
                nc.vector.tensor_copy(pack[:, C:2 * C], psS1[:])
                nc.scalar.copy(pack[0:1, 2 * C:3 * C], ps_ssq[:])

            # ---- combine partials: AllGather + local 3-add tree ----
            nc.sync.dma_start(out=cc_in[:], in_=pack[:])
            nc.gpsimd.collective_compute(
                "AllGather",
                ALU.bypass,
                replica_groups=[list(range(NCORES))],
                ins=[cc_in[:].opt()],
                outs=[cc_out[:].opt()],
            )
            nc.sync.dma_start(
                out=gath[:],
                in_=cc_out[:].rearrange("(r p) f -> p r f", p=128),
            )
            h = pq.tile([128, 4, SW], F32, tag="h4")
            nc.vector.tensor_add(h[:], gath[:, 0:4, :], gath[:, 4:8, :])
            h2 = pq.tile([128, 2, SW], F32, tag="h2")
            nc.vector.tensor_add(h2[:], h[:, 0:2, :], h[:, 2:4, :])
            nc.vector.tensor_add(stat[:], h2[:, 0, :], h2[:, 1, :])

            # shsc = -2*c0 * S^T in bf16 (phase-3 moving operand)
            nc.scalar.activation(shsc[:], stat[:, 0:2 * C], AF.Copy,
                                 bias=0.0, scale=consts[:, 1:2])

            # ---- phase 3 ----
            with tc.tile_pool(name="psum_p", bufs=1, space="PSUM") as qp:
                # ssqb = broadcast(Ssq) * c0  [128, 7] (single fp32 matmul)
                ps_bb = qp.tile([128, C], F32, tag="ps_bb")
                nc.tensor.matmul(ps_bb[:], lhsT=ones_row[:],
                                 rhs=stat[0:1, 2 * C:3 * C],
                                 start=True, stop=True)
                nc.scalar.activation(ssqb[:], ps_bb[:], AF.Copy,
                                     bias=0.0, scale=consts[:, 0:1])

                pPs = [qp.tile([128, C], F32, tag=f"pP{u}", name=f"pP{u}")
                       for u in range(T)]
                for u in range(T):
                    lo, hi = u * 128, (u + 1) * 128
                    nc.tensor.matmul(pPs[u][:], lhsT=xt0[:, lo:hi],
                                     rhs=shsc[:, 0:C], start=True, stop=False)
                    nc.tensor.matmul(pPs[u][:], lhsT=xt1[:, lo:hi],
                                     rhs=shsc[:, C:2 * C], start=False, stop=True)

                for u in range(T):
                    # q = c0*(Ssq - 2G)  (pP already carries -2*c0*G)
                    nc.vector.tensor_add(q_all[:, u, :], pPs[u][:], ssqb[:])
                    # own_q = sum_c mask*q (mask-mult with free row sum)
                    scr7 = pc.tile([128, C], F32, tag="scr7")
                    nc.vector.scalar_tensor_tensor(
                        out=scr7[:], in0=q_all[:, u, :], scalar=1.0,
                        in1=ytile[:, u, :],
                        op0=ALU.mult, op1=ALU.mult,
                        accum_out=own_all[:, u:u + 1],
                    )

                # z bias and own-column correction term, batched over tiles:
                # b2 = -(M/(M-1))*own_q - b_t2 ;  zo = -(1/(M-1))*own_q - b_t2
                nc.vector.scalar_tensor_tensor(
                    out=b2_all[:], in0=own_all[:], scalar=-float(M) / (M - 1),
                    in1=b_t2[:], op0=ALU.mult, op1=ALU.subtract,
                )
                nc.vector.scalar_tensor_tensor(
                    out=zo_all[:], in0=own_all[:], scalar=-1.0 / (M - 1),
                    in1=b_t2[:], op0=ALU.mult, op1=ALU.subtract,
                )

                for u in range(T):
                    ex = pc.tile([128, C], F32, tag="ex")
                    nc.scalar.activation(ex[:], q_all[:, u, :], AF.Exp,
                                         bias=b2_all[:, u:u + 1], scale=1.0,
                                         accum_out=se_all[:, u:u + 1])
                nc.scalar.activation(eo_all[:], zo_all[:], AF.Exp)

                # se_fix = se + 1 - exp(z_own);  L = relu(ln(se_fix))
                nc.vector.scalar_tensor_tensor(
                    out=se_fix[:], in0=se_all[:], scalar=1.0, in1=eo_all[:],
                    op0=ALU.add, op1=ALU.subtract,
                )
                nc.scalar.activation(lnse[:], se_fix[:], AF.Ln)
                nc.vector.tensor_scalar_max(accL[:], lnse[:], 0.0)

                # ---- reduce to scalar ----
                nc.vector.reduce_sum(accT[:], accL[:], axis=AX.X)
                ploss = ps_bb[0:1, 0:1]  # reuse the broadcast bank
                nc.tensor.matmul(ploss, lhsT=accT[:], rhs=ones_col[:],
                                 start=True, stop=True)
                nc.scalar.copy(out_s[:], ploss)
                nc.sync.dma_start(out=out_d[:, :], in_=out_s[:])

    nc.compile()
    return nc


_NC_CACHE = None


def _get_nc():
    global _NC_CACHE
    if _NC_CACHE is None:
        _NC_CACHE = build_program()
    return _NC_CACHE


def make_in_maps(embeddings, variance):
    X = np.ascontiguousarray(np.asarray(embeddings, dtype=np.float32))
    assert X.shape == (B, D), X.shape
    var = float(np.asarray(variance))

    labels = np.repeat(np.arange(C), M)  # reference ignores `target`
    Yfull = np.zeros((B, C), np.float32)
    Yfull[np.arange(B), labels] = 1.0

    c0 = -0.5 / (var * M)
    consts = np.zeros((128, 4), np.float32)
    consts[:, 0] = c0
    consts[:, 1] = -2.0 * c0
    consts[:, 2] = M * c0 / (M - 1)

    import ml_dtypes
    Xb = X.astype(ml_dtypes.bfloat16)

    in_maps = []
    for k in range(NCORES):
        s = slice(k * R, (k + 1) * R)
        in_maps.append({
            "xb": np.ascontiguousarray(Xb[s]),
            "xt": np.ascontiguousarray(Xb[s].T),
            "y": np.ascontiguousarray(Yfull[s]),
            "yb": np.ascontiguousarray(Yfull[s].astype(Xb.dtype)),
            "consts": consts,
        })
    return in_maps


def kernel(embeddings, target, variance):
    del target  # labels are balanced & class-sorted by construction (as in reference)
    nc = _get_nc()
    in_maps = make_in_maps(embeddings, variance)
    res = run_bass_kernel_spmd(nc, in_maps, list(range(NCORES)))
    total = 0.0
    for k in range(NCORES):
        total += float(res.results[k]["loss_part"][0, 0])
    return np.float32(total)


# revision 26
# speedup vs baseline: 2.4139x; 2.1682x over previous
"""Trainium2 Bass kernel for nn_KernelDensityLoss (KDE softmax loss).

Math: the reference's O(B^2*D) pairwise log-prob matrix collapses to
per-class sufficient statistics.  For row i and class c,

  A[i,c] = M*sq[i] + Ssq[c] - 2*G[i,c]     (G = X @ S^T, sq = ||x_i||^2,
                                            S_c = class sum, Ssq_c = class
                                            sum of squared norms)
  P[i,c] = -0.5*A[i,c] / (var*m_c)         (m_c = M-1 own class, M else)
  loss   = sum_i relu(logsumexp_c P[i,c] - P[i,own])

The Gaussian normalisation constant cancels in logsumexp - own.  With
c0 = -0.5/(var*M), the kernel computes q[i,c] = c0*(Ssq_c - 2*G[i,c]);
the per-row M*sq[i] term is dropped (a per-row additive constant K
cancels in z_c = P_c - (M/(M-1))*P_own except for a -c0*K/(M-1)
residue that folds into the per-row exp bias):

  z_c = q_c + b2_i,   b2_i = -(M/(M-1))*own_q_i - c0*M*sq_i/(M-1)
  se  = sum_c exp(z_c), own column's term replaced by its exact value
        1 via se += 1 - exp(z_own);   L_i = relu(ln(se)).

Distribution: phase 3 (per-row losses) is data-parallel over the 8
cores (896 rows each).  The tiny class stats are computed REDUNDANTLY
on every core from the full batch: on this runtime a cross-core
collective costs ~60us end-to-end (host-mediated trigger + rank-start
skew), far more than the ~12us of bf16 matmul it saves, so no
collective is used.  Stats stream the full batch as bf16 moving data
against a per-class one-hot stationary (labels are class-sorted, so
each 1024-row class spans 8 aligned 128-row tiles -> one stationary
per class chunk); squares for Ssq are computed on the fly, split
across vector/gpsimd/scalar engines.

All Exp inputs are pre-biased and batched into ONE [128, 7*7] Exp (plus
one [128,7] Exp for the own-column correction), so the activation
table is swapped once for the final Ln only.
NOTE: tensor_tensor_reduce crashes this runtime; scalar_tensor_tensor
(+accum_out) replaces it.
"""

import numpy as np

import concourse.bass as bass
import concourse.bacc as bacc
import concourse.mybir as mybir
import concourse.tile as tile
from concourse.bass_utils import run_bass_kernel_spmd

B = 7168      # total rows
C = 7         # classes
M = 1024      # rows per class
D = 256       # embedding dim
NCORES = 8
R = B // NCORES          # 896 rows per core
T = R // 128             # 7 row-tiles of 128 per core
TF = B // 128            # 56 tiles over the full batch
HC = TF // C             # 8 tiles per class chunk

F32 = mybir.dt.float32
BF16 = mybir.dt.bfloat16
AX = mybir.AxisListType
AF = mybir.ActivationFunctionType
ALU = mybir.AluOpType


def build_program():
    nc = bacc.Bacc(
        "TRN2",
        target_bir_lowering=False,
        debug=False,
        enable_asserts=True,
        num_devices=NCORES,
    )

    xf_d = nc.dram_tensor("xf", [B, D], BF16, kind="ExternalInput")
    xb_d = nc.dram_tensor("xb", [R, D], BF16, kind="ExternalInput")
    xt_d = nc.dram_tensor("xt", [D, R], BF16, kind="ExternalInput")
    y_d = nc.dram_tensor("y", [R, C], F32, kind="ExternalInput")
    ycls_d = nc.dram_tensor("ycls", [128, C * C], BF16, kind="ExternalInput")
    ident_d = nc.dram_tensor("ident", [C, C], F32, kind="ExternalInput")
    consts_d = nc.dram_tensor("consts", [128, 4], F32, kind="ExternalInput")
    out_d = nc.dram_tensor("loss_part", [1, 1], F32, kind="ExternalOutput")

    with tile.TileContext(nc) as tc:
        with (
            tc.tile_pool(name="persist", bufs=1) as pp,
            tc.tile_pool(name="sqscratch", bufs=2) as pq,
            tc.tile_pool(name="chunk", bufs=2) as pc,
        ):
            # ---- persistent tiles ----
            xfb = pp.tile([128, TF, D], BF16, tag="xfb")   # full batch
            xsb = pp.tile([128, TF, D], BF16, tag="xsb")   # its squares
            xb = pp.tile([128, T, D], BF16, tag="xb")      # own shard rows
            xt0 = pp.tile([128, R], BF16, tag="xt0")       # own shard, d 0:128
            xt1 = pp.tile([128, R], BF16, tag="xt1")       # own shard, d 128:256
            ytile = pp.tile([128, T, C], F32, tag="y")     # own one-hot mask
            ycls = pp.tile([128, C, C], BF16, tag="ycls")  # class one-hot bcast
            ident = pp.tile([C, C], F32, tag="ident")
            consts = pp.tile([128, 4], F32, tag="consts")
            sq = pp.tile([128, T], F32, tag="sq")          # own ||x_i||^2
            b_t2 = pp.tile([128, T], F32, tag="b_t2")      # sq*M*c0/(M-1)
            sA = pp.tile([7, 512], F32, tag="sA")
            sB = pp.tile([7, 512], F32, tag="sB")
            st7 = pp.tile([7, 256], F32, tag="st7")        # S  [c, d]
            s2 = pp.tile([7, 256], F32, tag="s2")          # S2 [c, d]
            s2h = pp.tile([128, 2 * C], F32, tag="s2h")    # S2^T halves
            shsc = pp.tile([128, 2 * C], BF16, tag="shsc")  # -2*c0*S^T
            ssq_row = pp.tile([1, C], F32, tag="ssq_row")
            ssqb = pp.tile([128, C], F32, tag="ssqb")      # c0*Ssq broadcast
            ones_row = pp.tile([1, 128], F32, tag="ones_row")
            ones_col = pp.tile([128, 1], F32, tag="ones_col")
            q_all = pp.tile([128, T, C], F32, tag="q_all")
            zq_all = pp.tile([128, T, C], F32, tag="zq_all")
            e_all = pp.tile([128, T, C], F32, tag="e_all")
            own_all = pp.tile([128, T], F32, tag="own_all")
            b2_all = pp.tile([128, T], F32, tag="b2_all")
            zo_all = pp.tile([128, T], F32, tag="zo_all")
            eo_all = pp.tile([128, T], F32, tag="eo_all")
            se_all = pp.tile([128, T], F32, tag="se_all")
            se_fix = pp.tile([128, T], F32, tag="se_fix")
            lnse = pp.tile([128, T], F32, tag="lnse")
            accL = pp.tile([128, T], F32, tag="accL")
            accT = pp.tile([128, 1], F32, tag="accT")
            out_s = pp.tile([1, 1], F32, tag="out_s")

            # ---- loads ----
            nc.sync.dma_start(out=consts[:], in_=consts_d[:, :])
            nc.sync.dma_start(out=ident[:], in_=ident_d[:, :])
            nc.sync.dma_start(out=ycls[:],
                              in_=ycls_d.ap().rearrange("p (a c) -> p a c", c=C))
            nc.sync.dma_start(out=ytile[:],
                              in_=y_d.ap().rearrange("(t p) c -> p t c", p=128))
            nc.sync.dma_start(out=xb[:],
                              in_=xb_d.ap().rearrange("(t p) d -> p t d", p=128))
            nc.sync.dma_start(out=xt0[:], in_=xt_d[0:128, :])
            nc.sync.dma_start(out=xt1[:], in_=xt_d[128:256, :])
            xf_r = xf_d.ap().rearrange("(a p) d -> p a d", p=128)
            for j in range(C):
                nc.sync.dma_start(out=xfb[:, HC * j:HC * j + HC, :],
                                  in_=xf_r[:, HC * j:HC * j + HC, :])

            nc.vector.memset(ones_row[:], 1.0)
            nc.gpsimd.memset(ones_col[:], 1.0)

            # ---- own-shard row norms (for the exp bias), fp32 accumulate ----
            for t in range(T):
                scr = pq.tile([128, D], F32, tag="sqscr")
                if t % 2 == 0:
                    nc.scalar.activation(scr[:], xb[:, t, :], AF.Square,
                                         bias=0.0, scale=1.0,
                                         accum_out=sq[:, t:t + 1])
                else:
                    nc.vector.tensor_mul(scr[:], xb[:, t, :], xb[:, t, :])
                    nc.vector.reduce_sum(sq[:, t:t + 1], scr[:], axis=AX.X)
            nc.vector.tensor_scalar_mul(b_t2[:], sq[:], consts[:, 2:3])

            # ---- full-batch class stats, one class chunk (8 tiles) at a
            # time: squares split across 3 engines, then 4+4 matmuls of 512
            # moving bf16 cols against the class one-hot stationary. ----
            with tc.tile_pool(name="psum_stat", bufs=1, space="PSUM") as qstat:
                psA = qstat.tile([7, 512], F32, tag="psA")
                psB = qstat.tile([7, 512], F32, tag="psB")
                for j in range(C):
                    g = HC * j
                    nc.vector.tensor_mul(xsb[:, g:g + 3, :], xfb[:, g:g + 3, :],
                                         xfb[:, g:g + 3, :])
                    nc.gpsimd.tensor_mul(xsb[:, g + 3:g + 6, :],
                                         xfb[:, g + 3:g + 6, :],
                                         xfb[:, g + 3:g + 6, :])
                    nc.scalar.activation(xsb[:, g + 6:g + 8, :],
                                         xfb[:, g + 6:g + 8, :], AF.Square,
                                         bias=0.0, scale=1.0)
                    st = (j == 0)
                    sp = (j == C - 1)
                    y_j = ycls[:, j, :]
                    for v in range(HC // 2):
                        nc.tensor.matmul(psA[:], lhsT=y_j,
                                         rhs=xfb[:, g + 2 * v:g + 2 * v + 2, :],
                                         start=st and v == 0,
                                         stop=sp and v == HC // 2 - 1)
                    for v in range(HC // 2):
                        nc.tensor.matmul(psB[:], lhsT=y_j,
                                         rhs=xsb[:, g + 2 * v:g + 2 * v + 2, :],
                                         start=st and v == 0,
                                         stop=sp and v == HC // 2 - 1)

                # fold even/odd halves -> S [7, 256], S2 [7, 256]
                # (evacuate first: a TensorTensor with two PSUM operands
                # fails the walrus verifier)
                nc.scalar.copy(sA[:], psA[:])
                nc.vector.tensor_copy(sB[:], psB[:])
                nc.vector.tensor_add(st7[:], sA[:, 0:256], sA[:, 256:512])
                nc.vector.tensor_add(s2[:], sB[:, 0:256], sB[:, 256:512])

            # ---- S^T, Ssq row, Ssq broadcast (PE transposes + ones matmuls)
            with tc.tile_pool(name="psum_t", bufs=2, space="PSUM") as qt:
                for hh in range(2):
                    tp = qt.tile([128, C], F32, tag="tp")
                    nc.tensor.transpose(tp[:], st7[:, 128 * hh:128 * hh + 128],
                                        ident[:])
                    # shsc = -2*c0 * S^T in bf16 (phase-3 moving operand)
                    nc.scalar.activation(shsc[:, C * hh:C * hh + C], tp[:],
                                         AF.Copy, bias=0.0,
                                         scale=consts[:, 1:2])
                    tq = qt.tile([128, C], F32, tag="tq")
                    nc.tensor.transpose(tq[:], s2[:, 128 * hh:128 * hh + 128],
                                        ident[:])
                    nc.vector.tensor_copy(s2h[:, C * hh:C * hh + C], tq[:])

                ps_sr = qt.tile([1, C], F32, tag="ps_sr")
                nc.tensor.matmul(ps_sr[:], lhsT=ones_col[:], rhs=s2h[:, 0:C],
                                 start=True, stop=False)
                nc.tensor.matmul(ps_sr[:], lhsT=ones_col[:], rhs=s2h[:, C:2 * C],
                                 start=False, stop=True)
                nc.scalar.copy(ssq_row[:], ps_sr[:])
                ps_bb = qt.tile([128, C], F32, tag="ps_bb")
                nc.tensor.matmul(ps_bb[:], lhsT=ones_row[:], rhs=ssq_row[:],
                                 start=True, stop=True)
                nc.scalar.activation(ssqb[:], ps_bb[:], AF.Copy,
                                     bias=0.0, scale=consts[:, 0:1])

            # ---- phase 3: per-row loss over own 896 rows ----
            with tc.tile_pool(name="psum_p", bufs=1, space="PSUM") as qp:
                pPs = [qp.tile([128, C], F32, tag=f"pP{u}", name=f"pP{u}")
                       for u in range(T)]
                ploss = qp.tile([1, 1], F32, tag="ploss")
                for u in range(T):
                    lo, hi = u * 128, (u + 1) * 128
                    nc.tensor.matmul(pPs[u][:], lhsT=xt0[:, lo:hi],
                                     rhs=shsc[:, 0:C], start=True, stop=False)
                    nc.tensor.matmul(pPs[u][:], lhsT=xt1[:, lo:hi],
                                     rhs=shsc[:, C:2 * C], start=False, stop=True)

                for u in range(T):
                    # q = c0*(Ssq - 2G)  (pP already carries -2*c0*G)
                    nc.vector.tensor_add(q_all[:, u, :], pPs[u][:], ssqb[:])
                    # own_q = sum_c mask*q (mask-mult with free row sum)
                    scr7 = pc.tile([128, C], F32, tag="scr7")
                    nc.vector.scalar_tensor_tensor(
                        out=scr7[:], in0=q_all[:, u, :], scalar=1.0,
                        in1=ytile[:, u, :],
                        op0=ALU.mult, op1=ALU.mult,
                        accum_out=own_all[:, u:u + 1],
                    )

                # b2 = -(M/(M-1))*own_q - b_t2 ;  zo = -(1/(M-1))*own_q - b_t2
                nc.vector.scalar_tensor_tensor(
                    out=b2_all[:], in0=own_all[:], scalar=-float(M) / (M - 1),
                    in1=b_t2[:], op0=ALU.mult, op1=ALU.subtract,
                )
                nc.vector.scalar_tensor_tensor(
                    out=zo_all[:], in0=own_all[:], scalar=-1.0 / (M - 1),
                    in1=b_t2[:], op0=ALU.mult, op1=ALU.subtract,
                )

                # z = q + b2 (bias broadcast per tile), then ONE batched Exp
                for u in range(T):
                    nc.vector.tensor_scalar_add(zq_all[:, u, :], q_all[:, u, :],
                                                b2_all[:, u:u + 1])
                nc.scalar.activation(e_all[:], zq_all[:], AF.Exp)
                nc.scalar.activation(eo_all[:], zo_all[:], AF.Exp)
                # se per tile (innermost-C reduction), then own-column fix
                nc.vector.reduce_sum(se_all[:].rearrange("p (t o) -> p t o", o=1),
                                     e_all[:], axis=AX.X)
                nc.vector.scalar_tensor_tensor(
                    out=se_fix[:], in0=se_all[:], scalar=1.0, in1=eo_all[:],
                    op0=ALU.add, op1=ALU.subtract,
                )
                nc.scalar.activation(lnse[:], se_fix[:], AF.Ln)
                nc.vector.tensor_scalar_max(accL[:], lnse[:], 0.0)

                # ---- reduce to scalar ----
                nc.vector.reduce_sum(accT[:], accL[:], axis=AX.X)
                nc.tensor.matmul(ploss[:], lhsT=accT[:], rhs=ones_col[:],
                                 start=True, stop=True)
                nc.scalar.copy(out_s[:], ploss[:])
                nc.sync.dma_start(out=out_d[:, :], in_=out_s[:])

    nc.compile()
    return nc


_NC_CACHE = None


def _get_nc():
    global _NC_CACHE
    if _NC_CACHE is None:
        _NC_CACHE = build_program()
    return _NC_CACHE


def make_in_maps(embeddings, variance):
    X = np.ascontiguousarray(np.asarray(embeddings, dtype=np.float32))
    assert X.shape == (B, D), X.shape
    var = float(np.asarray(variance))

    labels = np.repeat(np.arange(C), M)  # reference ignores `target`
    Yfull = np.zeros((B, C), np.float32)
    Yfull[np.arange(B), labels] = 1.0

    c0 = -0.5 / (var * M)
    consts = np.zeros((128, 4), np.float32)
    consts[:, 0] = c0
    consts[:, 1] = -2.0 * c0
    consts[:, 2] = M * c0 / (M - 1)

    import ml_dtypes
    Xb = X.astype(ml_dtypes.bfloat16)
    # class one-hot broadcast across partitions: ycls[:, c*C+j] = (j == c)
    ycls = np.zeros((128, C * C), ml_dtypes.bfloat16)
    for c in range(C):
        ycls[:, c * C + c] = 1.0

    in_maps = []
    for k in range(NCORES):
        s = slice(k * R, (k + 1) * R)
        in_maps.append({
            "xf": Xb,
            "xb": np.ascontiguousarray(Xb[s]),
            "xt": np.ascontiguousarray(Xb[s].T),
            "y": np.ascontiguousarray(Yfull[s]),
            "ycls": ycls,
            "ident": np.eye(C, dtype=np.float32),
            "consts": consts,
        })
    return in_maps


def kernel(embeddings, target, variance):
    del target  # labels are balanced & class-sorted by construction (as in reference)
    nc = _get_nc()
    in_maps = make_in_maps(embeddings, variance)
    res = run_bass_kernel_spmd(nc, in_maps, list(range(NCORES)))
    total = 0.0
    for k in range(NCORES):
        total += float(res.results[k]["loss_part"][0, 0])
    return np.float32(total)
